# revision 23
# baseline (speedup 1.0000x reference)
"""Trainium2 Bass kernel for causal self-attention with RoPE (mixed variant).

Sharding: data-parallel over batch, 2 cores (core b owns batch b end-to-end,
all 16 heads).  This minimizes per-call staged bytes — the dominant cost of
the dispatch path — versus replicating x across head-sharded cores and
summing partial projections on the host:
  per call: 2 x 12.6 MB input blob + 2 x 4 MB bf16 output = ~33 MB total
  (8-core head-parallel baseline staged ~120 MB).

Per-core device pipeline (all matmuls bf16 with f32 PSUM accumulate):
  A) qk^T = W_qk^T @ x^T   -> [d, t] layout; RoPE applied in [d, t] via
     pair-swapped copy (even/odd partition swap) + cos/sin tables (NEFF
     consts).
  B) v = x @ W_v           -> [t, d] layout, with a ones-column appended
     per head (softmax denominator trick).
  C) per head: S^T tiles = k^T.T @ q^T (K=64), causal mask added via a
     constant matmul accumulate, exp on ScalarE (scale=1/8 fused),
     P^T @ [V|1] accumulates O'^T = [O^T; denom] in PSUM.  Normalize by
     1/denom (broadcast via gpsimd) -> O^T bf16.
  D) y = O^T.T @ W_p rows accumulated over all 8 row-groups, bf16 out.

All inputs ship as ONE bf16 blob per core ([C, 6144] = x^T | w_q | w_k |
w_v | w_proj); cos/sin/mask tables are inline Const tensors baked into the
NEFF (loaded once at model load, zero per-call cost).
"""

import numpy as np
import ml_dtypes
from contextlib import ExitStack

B, T, C = 2, 2048, 1024
NH, HD = 16, 64
NCORES = 2            # one core per batch
NG = NH // 2          # 8 groups of 128 q/k rows (2 heads each)
CT = C // 128         # 8 contraction tiles
NTT = T // 128        # 16 t-tiles
MASK_NEG = -30000.0

# blob column offsets
XT_OFF = 0            # x^T        [C, T]
WQK_OFF = T           # w_q | w_k  [C, 2C]
WV_OFF = T + 2 * C    # w_v        [C, C]
WP_OFF = T + 3 * C    # w_proj     [C, C]
BLOB_COLS = T + 4 * C  # 6144

bf16 = ml_dtypes.bfloat16

_CACHE: dict = {}


def _make_tables():
    """cos/sin tables ([128, T], two 64-row head copies) and mask consts."""
    hd = HD
    inv_freq = 1.0 / (10000.0 ** (np.arange(0, hd, 2, dtype=np.float64) / hd))
    t = np.arange(T, dtype=np.float64)
    emb = t[:, None] * np.concatenate([inv_freq, inv_freq])[None, :]  # [T, 64]
    cos = np.cos(emb).T.astype(np.float32)       # [64, T]
    sin = np.sin(emb).T.astype(np.float32)
    sign = np.where(np.arange(hd) % 2 == 0, -1.0, 1.0).astype(np.float32)
    sin = sin * sign[:, None]
    cos128 = np.concatenate([cos, cos], axis=0).astype(bf16)   # [128, T]
    sin128 = np.concatenate([sin, sin], axis=0).astype(bf16)
    ii = np.arange(128)
    mA = (ii[:, None] < ii[None, :]).astype(bf16)              # A[c, m] = c < m
    mB = (MASK_NEG * np.eye(128)).astype(bf16)
    return cos128, sin128, mA, mB


def _emit(tc, nc, mybir, bass, ctx):
    dt = mybir.dt
    f32, b16 = dt.float32, dt.bfloat16
    AF = mybir.ActivationFunctionType

    blob_d = nc.dram_tensor("blob", [C, BLOB_COLS], b16, kind="ExternalInput")
    y_d = nc.dram_tensor("y", [T, C], b16, kind="ExternalOutput")

    cos128, sin128, mA, mB = _make_tables()
    cos_d = nc.inline_tensor(np.asarray(cos128), name="cosT")
    sin_d = nc.inline_tensor(np.asarray(sin128), name="sinT")
    mA_d = nc.inline_tensor(np.asarray(mA), name="mA")
    mB_d = nc.inline_tensor(np.asarray(mB), name="mB")

    const = ctx.enter_context(tc.tile_pool(name="const", bufs=1))
    work = ctx.enter_context(tc.tile_pool(name="work", bufs=1))

    # ---- resident SBUF tiles (DMAs issued inside phase A/B, ordered so
    # the first matmul's dependencies land first) ----
    cos_sb = const.tile([128, T], b16, tag="cos")
    sin_sb = const.tile([128, T], b16, tag="sin")
    mA_sb = const.tile([128, 128], b16, tag="mA")
    mB_sb = const.tile([128, 128], b16, tag="mB")

    # rope outputs: [d, t] bf16, NG grp-tiles each (grp = 2 heads = 128 rows)
    q_sb = work.tile([128, NG, T], b16, tag="q")
    k_sb = work.tile([128, NG, T], b16, tag="k")
    # v in [t, d] layout with per-head ones column: [t-tile, head, 65]
    v_sb = work.tile([128, NTT, NH, HD + 1], b16, tag="v")

    nc.gpsimd.memset(v_sb[:], 1.0)  # ones columns (v cols overwritten below)

    # ---- phase A: qk^T matmuls + rope;  phase B: v matmuls ----
    with (
        tc.tile_pool(name="xw", bufs=1) as xw_pool,
        tc.tile_pool(name="wstream", bufs=3) as w_pool,
        tc.tile_pool(name="qk_ps", bufs=3, space="PSUM") as qk_pool,
        tc.tile_pool(name="v_ps", bufs=2, space="PSUM") as v_pool,
        tc.tile_pool(name="rope", bufs=2) as rope_pool,
    ):
        # w_q|w_k in four 512-col chunks (tag-shared slots; chunk c covers
        # dtiles 4c..4c+3), prefetched ahead of use.  Chunk 0 is split
        # per-c-tile and interleaved with x^T so the first matmul's
        # dependencies (w0[ci=0], xT[ci=0]) arrive first.
        xt_sb = xw_pool.tile([128, CT, T], b16, tag="xt")
        wt0 = w_pool.tile([128, CT, 512], b16, tag="w")
        w_chunks = [wt0]
        nc.sync.dma_start(xt_sb[:, 0, 0:512],
                          blob_d.ap()[0:128, XT_OFF:XT_OFF + 512])
        nc.sync.dma_start(w_chunks[0][:, 0, :],
                          blob_d.ap()[0:128, WQK_OFF:WQK_OFF + 512])
        nc.sync.dma_start(xt_sb[:, 0, 512:T],
                          blob_d.ap()[0:128, XT_OFF + 512:T])
        for i in range(1, CT):
            nc.sync.dma_start(
                xt_sb[:, i, :], blob_d.ap()[i * 128:(i + 1) * 128, XT_OFF:T])
            nc.sync.dma_start(
                w_chunks[0][:, i, :],
                blob_d.ap()[i * 128:(i + 1) * 128, WQK_OFF:WQK_OFF + 512])
        nc.sync.dma_start(cos_sb[:], cos_d.ap())
        nc.sync.dma_start(sin_sb[:], sin_d.ap())
        for wc in range(1, 4):
            wt = w_pool.tile([128, CT, 512], b16, tag="w")
            c0 = WQK_OFF + wc * 512
            nc.sync.dma_start(
                wt[:], blob_d.ap()[:, c0:c0 + 512].rearrange(
                    "(a p) d -> p a d", p=128))
            w_chunks.append(wt)
        nc.sync.dma_start(mA_sb[:], mA_d.ap())
        nc.sync.dma_start(mB_sb[:], mB_d.ap())

        for dtile in range(2 * NG):  # 8 q grp-tiles then 8 k grp-tiles
            is_q = dtile < NG
            grp = dtile % NG
            wt = w_chunks[dtile // 4]
            wcol = (dtile % 4) * 128
            for half in range(2):  # [128, 1024] halves for psum double-buffer
                h0 = half * (T // 2)
                ps = qk_pool.tile([128, T // 2], f32, tag="qkps")
                for j in range(2):
                    for ci in range(CT):
                        nc.tensor.matmul(
                            ps[:, j * 512:(j + 1) * 512],
                            wt[:, ci, wcol:wcol + 128],
                            xt_sb[:, ci, h0 + j * 512:h0 + (j + 1) * 512],
                            start=(ci == 0),
                            stop=(ci == CT - 1),
                        )
                    # rope per 512-col psum bank (bank-level deps let this
                    # overlap the next chunk's matmuls)
                    csl = slice(h0 + j * 512, h0 + (j + 1) * 512)
                    psl = slice(j * 512, (j + 1) * 512)
                    # evacuate to bf16 SBUF (ScalarE, closer to PSUM)
                    raw = rope_pool.tile([128, 512], b16, tag="raw")
                    nc.scalar.copy(raw[:], ps[:, psl])
                    # pair-swap partitions (d even<->odd): 32-way shuffle
                    shuf = rope_pool.tile([128, 512], b16, tag="shuf")
                    nc.vector.stream_shuffle(shuf[:], raw[:],
                                             [i ^ 1 for i in range(32)])
                    # rope: out = raw*cos + shuf*sin'
                    t1 = rope_pool.tile([128, 512], b16, tag="t1")
                    nc.vector.tensor_mul(t1[:], raw[:], cos_sb[:, csl])
                    t2 = rope_pool.tile([128, 512], b16, tag="t2")
                    nc.vector.tensor_mul(t2[:], shuf[:], sin_sb[:, csl])
                    dst = (q_sb if is_q else k_sb)
                    nc.vector.tensor_add(dst[:, grp, csl], t1[:], t2[:])

        # v weight chunks (reuse the streaming tag)
        wv_chunks = []
        for wc in range(2):
            wt = w_pool.tile([128, CT, 512], b16, tag="w")
            c0 = WV_OFF + wc * 512
            nc.sync.dma_start(
                wt[:], blob_d.ap()[:, c0:c0 + 512].rearrange(
                    "(a p) d -> p a d", p=128))
            wv_chunks.append(wt)

        # phase B: v in [t, d] layout
        for tt in range(NTT):
            for vc in range(2):  # 512 v-cols (8 heads) per psum tile
                vps = v_pool.tile([128, 512], f32, tag="vps")
                for ci in range(CT):
                    nc.tensor.matmul(
                        vps[:],
                        xt_sb[:, ci, tt * 128:(tt + 1) * 128],
                        wv_chunks[vc][:, ci, :],
                        start=(ci == 0),
                        stop=(ci == CT - 1),
                    )
                nc.scalar.copy(
                    v_sb[:, tt, vc * 8:(vc + 1) * 8, 0:HD],
                    vps[:].rearrange("p (h d) -> p h d", h=8),
                )

    # attention outputs O^T (normalized), [d, t], NG grp-tiles
    o_sb = work.tile([128, NG, T], b16, tag="o")

    # w_proj loads here (A/B pools freed) and hides under phase C; own pool
    # so its SBUF space is not reserved during phases A/B
    wp_pool = ctx.enter_context(tc.tile_pool(name="wpp", bufs=1))
    wp_sb = wp_pool.tile([128, CT, C], b16, tag="wp")
    nc.sync.dma_start(
        wp_sb[:],
        blob_d.ap()[:, WP_OFF:WP_OFF + C].rearrange("(a p) d -> p a d", p=128),
    )

    # ---- phase C: attention per head ----
    with (
        tc.tile_pool(name="o_ps", bufs=2, space="PSUM") as o_pool,
        tc.tile_pool(name="s_ps", bufs=3, space="PSUM") as s_pool,
        tc.tile_pool(name="p_sb", bufs=4) as p_pool,
        tc.tile_pool(name="r_sb", bufs=4) as r_pool,
    ):
        # Head PAIRS share each S psum tile: head0 in cols 0:512 (PE rows
        # 0:63), head1 in cols 512:1024 (PE rows 64:127).  The two K=64
        # S matmuls auto-derive tile_position (0,0)/(64,0) from their
        # base partitions and run CONCURRENTLY in disjoint PE row strips,
        # halving S time vs sequential heads.
        for g in range(NG):
            for j in range(4):  # 512-query windows
                w0 = j * 512
                ops0 = o_pool.tile([65, 512], f32, tag="ops")
                ops1 = o_pool.tile([65, 512], f32, tag="ops")
                ilim = 4 * j + 4
                for i in range(ilim):
                    off = max(0, 128 * i - w0)  # first valid col in window
                    diag = 4 * j <= i  # key tile overlaps the diagonal
                    sps = s_pool.tile([128, 1024], f32, tag="sps")
                    for h in range(2):
                        base = 64 * h
                        nc.tensor.matmul(
                            sps[:, h * 512 + off:(h + 1) * 512],
                            k_sb[base:base + 64, g, i * 128:(i + 1) * 128],
                            q_sb[base:base + 64, g, w0 + off:w0 + 512],
                            start=True,
                            stop=not diag,
                        )
                    if diag:
                        d0 = 128 * i - w0  # tri-block col within window
                        for h in range(2):
                            nc.tensor.matmul(
                                sps[:, h * 512 + d0:h * 512 + d0 + 128],
                                mA_sb[:],
                                mB_sb[:],
                                start=False,
                                stop=True,
                            )
                    psb = p_pool.tile([128, 1024], b16, tag="psb")
                    if off == 0:
                        # one instruction covers both heads' columns
                        nc.scalar.activation(psb[:], sps[:], AF.Exp,
                                             scale=0.125)
                    else:
                        for h in range(2):
                            nc.scalar.activation(
                                psb[:, h * 512 + off:(h + 1) * 512],
                                sps[:, h * 512 + off:(h + 1) * 512],
                                AF.Exp, scale=0.125,
                            )
                    for h, ops in ((0, ops0), (1, ops1)):
                        nc.tensor.matmul(
                            ops[:, off:512],
                            v_sb[:, i, 2 * g + h, :],
                            psb[:, h * 512 + off:(h + 1) * 512],
                            start=(i == 0),
                            stop=(i == ilim - 1),
                        )
                # normalize this 512-col window: O^T * (1/denom).  Evacuate
                # PSUM with a fast ACT copy first so the o_ps bank frees
                # for the next window while DVE/gpsimd normalize from SBUF.
                for h, ops in ((0, ops0), (1, ops1)):
                    base = 64 * h
                    osb = r_pool.tile([65, 512], dt.float32, tag="osb")
                    nc.scalar.copy(osb[:], ops[:])
                    rec = r_pool.tile([1, 512], dt.float32, tag="rec")
                    nc.vector.reciprocal(rec[:], osb[64:65, :])
                    rrep = r_pool.tile([64, 512], dt.float32, tag="rrep")
                    nc.gpsimd.partition_broadcast(rrep[:], rec[:])
                    nc.vector.tensor_mul(
                        o_sb[base:base + 64, g, w0:w0 + 512],
                        osb[0:64, :], rrep[:])

    # ---- phase D: projection y = O^T.T @ W_p (accumulate over groups) ----
    with (
        tc.tile_pool(name="y_ps", bufs=4, space="PSUM") as y_pool,
        tc.tile_pool(name="y_sb", bufs=4) as ysb_pool,
    ):
        for tt in range(NTT):
            for cc in range(2):
                yps = y_pool.tile([128, 512], f32, tag="yps")
                for grp in range(NG):
                    nc.tensor.matmul(
                        yps[:],
                        o_sb[:, grp, tt * 128:(tt + 1) * 128],
                        wp_sb[:, grp, cc * 512:(cc + 1) * 512],
                        start=(grp == 0),
                        stop=(grp == NG - 1),
                    )
                ysb = ysb_pool.tile([128, 512], b16, tag="ysb")
                # alternate ACT/DVE so neither engine gates the PE
                if cc == 0:
                    nc.scalar.copy(ysb[:], yps[:])
                else:
                    nc.vector.tensor_copy(ysb[:], yps[:])
                nc.sync.dma_start(
                    y_d.ap()[tt * 128:(tt + 1) * 128, cc * 512:(cc + 1) * 512],
                    ysb[:],
                )


def build_program():
    if "nc" in _CACHE:
        return _CACHE["nc"]
    import concourse.bass as bass
    import concourse.bacc as bacc
    import concourse.tile as tile
    import concourse.mybir as mybir

    nc = bacc.Bacc("TRN2", target_bir_lowering=False, debug=False,
                   enable_asserts=True)
    with tile.TileContext(nc) as tc:
        with ExitStack() as ctx:
            _emit(tc, nc, mybir, bass, ctx)
    nc.compile()
    _CACHE["nc"] = nc
    return nc


def make_in_maps(x, w_qkv, w_proj):
    wq = w_qkv[:, 0:C].astype(bf16)
    wk = w_qkv[:, C:2 * C].astype(bf16)
    wv = w_qkv[:, 2 * C:3 * C].astype(bf16)
    wp = w_proj.astype(bf16)
    in_maps = []
    for b in range(B):
        xT = np.ascontiguousarray(x[b].T).astype(bf16)
        blob = np.concatenate([xT, wq, wk, wv, wp], axis=1)
        in_maps.append({"blob": blob})
    return in_maps


def _backend_reset(delay):
    import time
    time.sleep(delay)
    try:
        import jax.extend.backend
        jax.extend.backend.clear_backends()
    except Exception:
        pass


def kernel(x, w_qkv, w_proj):
    """Retries cover two intermittent device-fault modes seen on this
    setup: a raised NRT_EXEC_UNIT_UNRECOVERABLE on fresh-process starts,
    and silently corrupted (non-finite) output.  The true output is
    bounded, so any NaN/Inf means a faulted execution."""
    from concourse import bass_utils
    nc = build_program()
    in_maps = make_in_maps(np.asarray(x, dtype=np.float32),
                           np.asarray(w_qkv, dtype=np.float32),
                           np.asarray(w_proj, dtype=np.float32))
    out = np.empty((B, T, C), dtype=np.float32)
    last_err = None
    for attempt in range(4):
        try:
            res = bass_utils.run_bass_kernel_spmd(
                nc, in_maps, list(range(NCORES)))
            for b in range(B):
                out[b] = res.results[b]["y"].astype(np.float32)
        except Exception as e:  # transient axon/NRT device wedge
            last_err = e
            _backend_reset(2.0 * (attempt + 1))
            continue
        if np.isfinite(out).all():
            return out
        last_err = RuntimeError("non-finite kernel output (device fault)")
        _backend_reset(2.0 * (attempt + 1))
    raise last_err
